# revision 10
# baseline (speedup 1.0000x reference)
"""MixMultiHeadAttention Trainium2 kernel.

Sharding: tensor-parallel over the 8 heads. Device h computes head h for all
batches and the partial out-projection ctx_h @ out_w[64h:64h+64, :]; the host
sums the 8 partials (the "all-reduce") and adds out_b.

Math per (batch b, head h), with Q0 = 1536, LQ = 576, L = 2112:
  Kt[d, k]  = sum_i x[b, k, i] wk[i, 64h+d]          (shared for k < 2048,
  Vt[d, k]    likewise                                per-token nw for k >= 2048)
  Qt[d, q]  = sum_i x[b, Q0+q, i] wq[i, 64h+d]
  S^T[k, q] = Kt[:, k] . Qt[:, q]
  P^T       = exp(S^T / 8) * causal(k - 1536 <= q)    (no max subtraction: |S/8| ~ 1)
  V_aug[k]  = [V[k] * m[k], m[k]]   (m = padding mask; folds pad mask + denominator)
  ctxa^T    = V_aug^T @ P^T         -> rows 0:64 unnormalized ctx^T, row 64 = denom
  out[q, :] = (ctx^T[:, q] . ow) / denom[q]

Layouts are prepared on the host (x pre-transposed to [i, t]) so every matmul
has its contraction dim on partitions.
"""

import os
import numpy as np
import ml_dtypes

import concourse.bass as bass
import concourse.mybir as mybir
from concourse.tile import TileContext
from concourse.bass_utils import run_bass_kernel_spmd
from concourse.vector_clock import ScopedClock

B, D, H, L_NS, L_S = 8, 512, 8, 64, 2048
HD = D // H            # 64
L = L_S + L_NS         # 2112
L_SO = 512
LQ = L_SO + L_NS       # 576
Q0 = L_S - L_SO        # 1536
NKC = 17               # key chunks of 128 (16 full + one 64)
NQC = 5                # query chunks of 128 (4 full + one 64)
NIC = D // 128         # 4 contraction chunks

F32 = mybir.dt.float32
BF16 = mybir.dt.bfloat16
USE_BF16 = os.environ.get("KERNEL_DTYPE", "bf16") == "bf16"
MM_DT = BF16 if USE_BF16 else F32
NP_DT = ml_dtypes.bfloat16 if USE_BF16 else np.float32

# ---------------------------------------------------------------------------
# Workaround: this walrus build allows at most 1 sem wait on the TileContext
# exit Drain; spread the remaining waits across preceding SP nops.
# ---------------------------------------------------------------------------
from concourse import tile as _tile_mod


def _patched_drain_and_barrier(self, tick_clock, wait_clock):
    nc = self.nc
    nops = [nc.sync.nop() for _ in range(48)]
    drain_inst = nc.sync.drain()
    wait_clock.add_sem_waits(
        drain_inst.ins, ScopedClock({None: tick_clock.global_clock})
    )
    si = drain_inst.ins.sync_info
    if si is not None and si.on_wait and len(si.on_wait) > 1:
        waits = list(si.on_wait)
        extra, keep = waits[:-1], waits[-1:]
        assert len(extra) <= len(nops), f"need {len(extra)} spare nops"
        for w, n in zip(extra, nops):
            n.ins.sync_info = mybir.SyncInfo(on_wait=[w], on_update=[])
        drain_inst.ins.sync_info = mybir.SyncInfo(
            on_wait=keep, on_update=list(si.on_update or [])
        )
    nc.all_engine_barrier()
    assert self.sems is not None
    popped = nc._tile_sem_poison_stack.pop()
    assert popped is self._sem_poison
    nc.clear_and_free_semaphores(list(self.sems.allocated().values()))
    nc.all_engine_barrier()


_tile_mod.TileContext._drain_and_barrier = _patched_drain_and_barrier


def _split_multi_waits(nc, max_waits=1):
    """This walrus build rejects instructions carrying more than one sem wait.
    Hoist extra waits onto standalone same-engine EventSemaphore (pure wait)
    instructions inserted just before the offending instruction."""
    ctr = 0
    for f in nc.m.functions:
        for bb in f.blocks:
            new = []
            for inst in bb.instructions:
                si = getattr(inst, "sync_info", None)
                waits = list(si.on_wait) if si is not None and si.on_wait else []
                if len(waits) > max_waits:
                    for w in waits[:-max_waits]:
                        ctr += 1
                        nop = mybir.InstEventSemaphore(
                            name=f"W-split-{ctr}", ins=[], outs=[]
                        )
                        nop.engine = inst.engine
                        nop.sync_info = mybir.SyncInfo(on_wait=[w], on_update=[])
                        new.append(nop)
                    inst.sync_info = mybir.SyncInfo(
                        on_wait=waits[-max_waits:],
                        on_update=list(si.on_update or []),
                    )
                new.append(inst)
            bb.instructions = new
    return ctr


def build_nc():
    nc = bass.Bass()
    xt_d = nc.dram_tensor("xt", [B, NIC, 128, L], MM_DT, kind="ExternalInput")
    xns_d = nc.dram_tensor("xns", [128, NIC * L_NS * 8], MM_DT, kind="ExternalInput")
    wkv_d = nc.dram_tensor("wkv", [128, NIC * 128], MM_DT, kind="ExternalInput")
    wq_d = nc.dram_tensor("wq", [128, NIC * 64], MM_DT, kind="ExternalInput")
    nw_d = nc.dram_tensor("nw", [16, 128, 3072], MM_DT, kind="ExternalInput")
    ow_d = nc.dram_tensor("ow", [64, 512], F32, kind="ExternalInput")
    padf_d = nc.dram_tensor("padf", [128, B * NKC], F32, kind="ExternalInput")
    out_d = nc.dram_tensor("out_p", [B, LQ, D], F32, kind="ExternalOutput")

    with TileContext(nc) as tc:
        with (
            tc.tile_pool(name="const", bufs=1) as cp,
            tc.tile_pool(name="nwp", bufs=3) as nwp,
            tc.tile_pool(name="nsout", bufs=1) as nso,
            tc.tile_pool(name="xp", bufs=2) as xp,
            tc.tile_pool(name="kvq", bufs=2) as kvq,
            tc.tile_pool(name="att", bufs=3) as att,
            tc.tile_pool(name="outs", bufs=3) as outs,
            tc.tile_pool(name="ps1", bufs=2, space="PSUM") as ps1,
            tc.tile_pool(name="pss", bufs=2, space="PSUM") as pss,
            tc.tile_pool(name="psc", bufs=1, space="PSUM") as psc,
        ):
            # ---- constants / persistent tiles ----
            wkv_t = cp.tile([128, NIC * 128], MM_DT, tag="wkv")
            nc.sync.dma_start(wkv_t[:], wkv_d[:])
            wq_t = cp.tile([128, NIC * 64], MM_DT, tag="wq")
            nc.sync.dma_start(wq_t[:], wq_d[:])
            ow_t = cp.tile([64, 512], F32, tag="ow")
            nc.sync.dma_start(ow_t[:], ow_d[:])
            padf_t = cp.tile([128, B * NKC], F32, tag="padf")
            nc.sync.dma_start(padf_t[:], padf_d[:])
            xns_t = cp.tile([128, NIC * L_NS * 8], MM_DT, tag="xns")
            nc.sync.dma_start(xns_t[:], xns_d[:])

            # identity [64, 64] replicated in both partition halves (the
            # transpose lhsT sits at base partition 64, and matmul requires
            # lhsT/rhs base partitions to match)
            id64 = cp.tile([128, 64], MM_DT, tag="id64")
            nc.gpsimd.memset(id64[0:64, :], 1.0)
            nc.gpsimd.affine_select(
                out=id64[0:64, :], in_=id64[0:64, :],
                compare_op=mybir.AluOpType.is_equal, fill=0.0,
                base=0, pattern=[[-1, 64]], channel_multiplier=1,
            )
            nc.sync.dma_start(id64[64:128, :], id64[0:64, :])
            ones1 = cp.tile([128, 1], F32, tag="ones1")
            nc.gpsimd.memset(ones1[:], 1.0)

            # ---- ns (per-token) projections for all batches ----
            # qkns[0:64, 8n+b] = Q_ns, qkns[64:128, 8n+b] = K_ns, vns[:, 8n+b] = V_ns
            qkns = nso.tile([128, L_NS * 8], MM_DT, tag="qkns")
            vns = nso.tile([64, L_NS * 8], MM_DT, tag="vns")
            for g in range(16):
                nw_t = nwp.tile([128, 3072], MM_DT, tag="nw")
                nc.sync.dma_start(nw_t[:], nw_d[g, :, :])
                for nl in range(4):
                    n = 4 * g + nl
                    psA = ps1.tile([128, 8], F32, tag="mm1")
                    psB = ps1.tile([64, 8], F32, tag="mm1")
                    for c in range(NIC):
                        rhs = xns_t[:, c * 512 + 8 * n : c * 512 + 8 * n + 8]
                        base = 768 * nl + 192 * c
                        nc.tensor.matmul(
                            psA[:], nw_t[:, base : base + 128], rhs,
                            start=(c == 0), stop=(c == NIC - 1),
                        )
                        nc.tensor.matmul(
                            psB[:], nw_t[:, base + 128 : base + 192], rhs,
                            start=(c == 0), stop=(c == NIC - 1),
                        )
                    nc.vector.tensor_copy(qkns[:, 8 * n : 8 * n + 8], psA[:])
                    nc.scalar.copy(vns[:, 8 * n : 8 * n + 8], psB[:])

            qkns_v = qkns[:].rearrange("p (n b) -> p b n", b=8)
            vns_v = vns[:].rearrange("p (n b) -> p b n", b=8)

            # ---- main per-batch loop ----
            for b in range(B):
                # load x^T for this batch: [128, NIC*L] (chunk-major cols)
                xt_t = xp.tile([128, NIC * L], MM_DT, tag="xt")
                for c in range(NIC):
                    nc.sync.dma_start(xt_t[:, c * L : (c + 1) * L], xt_d[b, c, :, :])

                # shared K|V projection -> kvt[0:64]=Kt, kvt[64:128]=Vt
                kvt = kvq.tile([128, L], MM_DT, tag="kvt")
                for tb in range(4):
                    kv_ps = ps1.tile([128, 512], F32, tag="mm1")
                    for c in range(NIC):
                        nc.tensor.matmul(
                            kv_ps[:],
                            wkv_t[:, 128 * c : 128 * (c + 1)],
                            xt_t[:, c * L + 512 * tb : c * L + 512 * (tb + 1)],
                            start=(c == 0), stop=(c == NIC - 1),
                        )
                    nc.scalar.copy(kvt[:, 512 * tb : 512 * (tb + 1)], kv_ps[:])

                # shared Q projection (tokens Q0..Q0+512)
                qt = kvq.tile([64, LQ], MM_DT, tag="qt")
                q_ps = ps1.tile([64, 512], F32, tag="mm1")
                for c in range(NIC):
                    nc.tensor.matmul(
                        q_ps[:],
                        wq_t[:, 64 * c : 64 * (c + 1)],
                        xt_t[:, c * L + Q0 : c * L + Q0 + 512],
                        start=(c == 0), stop=(c == NIC - 1),
                    )
                nc.scalar.copy(qt[:, 0:512], q_ps[:])
                # splice in the ns projections for this batch
                nc.vector.tensor_copy(qt[:, 512:LQ], qkns_v[0:64, b, :])
                nc.vector.tensor_copy(kvt[0:64, L_S:L], qkns_v[64:128, b, :])
                nc.vector.tensor_copy(kvt[64:128, L_S:L], vns_v[:, b, :])

                # V_aug: per key chunk [128, 65]: cols 0:64 = V*m, col 64 = m
                vaug = kvq.tile([128, NKC * 65], MM_DT, tag="vaug")
                for kc in range(NKC):
                    w = 128 if kc < 16 else 64
                    vtr_ps = ps1.tile([128, 64], MM_DT, tag="mm1")
                    nc.tensor.transpose(
                        vtr_ps[0:w, :], kvt[64:128, 128 * kc : 128 * kc + w], id64[64:128, :]
                    )
                    mcol = padf_t[0:w, NKC * b + kc : NKC * b + kc + 1]
                    nc.vector.tensor_scalar_mul(
                        vaug[0:w, 65 * kc : 65 * kc + 64], vtr_ps[0:w, :], mcol
                    )
                    nc.vector.tensor_copy(
                        vaug[0:w, 65 * kc + 64 : 65 * kc + 65], mcol
                    )

                # attention: S^T chunks -> exp -> causal mask -> PV accumulate
                ctx_ps = psc.tile([65, LQ], F32, tag="ctx")
                for kc in range(NKC):
                    w = 128 if kc < 16 else 64
                    s_ps = pss.tile([128, LQ], F32, tag="s")
                    nc.tensor.matmul(
                        s_ps[0:w, 0:512],
                        kvt[0:64, 128 * kc : 128 * kc + w],
                        qt[:, 0:512], start=True, stop=True,
                    )
                    nc.tensor.matmul(
                        s_ps[0:w, 512:LQ],
                        kvt[0:64, 128 * kc : 128 * kc + w],
                        qt[:, 512:LQ], start=True, stop=True,
                    )
                    pt = att.tile([128, LQ], MM_DT, tag="pt")
                    nc.scalar.activation(
                        pt[0:w, :], s_ps[0:w, :],
                        mybir.ActivationFunctionType.Exp, scale=0.125,
                    )
                    if kc >= 12:
                        nc.gpsimd.affine_select(
                            out=pt[0:w, :], in_=pt[0:w, :],
                            compare_op=mybir.AluOpType.is_ge, fill=0.0,
                            base=-128 * (kc - 12),
                            pattern=[[1, LQ]], channel_multiplier=-1,
                        )
                    nc.tensor.matmul(
                        ctx_ps[:, 0:512],
                        vaug[0:w, 65 * kc : 65 * kc + 65],
                        pt[0:w, 0:512],
                        start=(kc == 0), stop=(kc == NKC - 1),
                    )
                    nc.tensor.matmul(
                        ctx_ps[:, 512:LQ],
                        vaug[0:w, 65 * kc : 65 * kc + 65],
                        pt[0:w, 512:LQ],
                        start=(kc == 0), stop=(kc == NKC - 1),
                    )

                ctxt = outs.tile([65, LQ], F32, tag="ctxt")
                nc.scalar.copy(ctxt[:], ctx_ps[:])

                # denominators -> [q, 1] via K=1 matmul transpose, then 1/x
                rec = outs.tile([128, NQC], F32, tag="rec")
                for qc in range(NQC):
                    wq_ = 128 if qc < 4 else 64
                    dn_ps = ps1.tile([128, 1], F32, tag="mm1")
                    nc.tensor.matmul(
                        dn_ps[0:wq_, :],
                        ctxt[64:65, 128 * qc : 128 * qc + wq_],
                        ones1[64:65, :], start=True, stop=True,
                    )
                    nc.vector.reciprocal(rec[0:wq_, qc : qc + 1], dn_ps[0:wq_, :])

                # out-projection + normalization + store
                for qc in range(NQC):
                    wq_ = 128 if qc < 4 else 64
                    o_ps = ps1.tile([128, 512], F32, tag="mm1")
                    nc.tensor.matmul(
                        o_ps[0:wq_, :],
                        ctxt[0:64, 128 * qc : 128 * qc + wq_],
                        ow_t[:], start=True, stop=True,
                    )
                    o_sb = outs.tile([128, 512], F32, tag="osb")
                    nc.vector.tensor_scalar_mul(
                        o_sb[0:wq_, :], o_ps[0:wq_, :], rec[0:wq_, qc : qc + 1]
                    )
                    nc.sync.dma_start(
                        out_d[b, 128 * qc : 128 * qc + wq_, :], o_sb[0:wq_, :]
                    )
    _split_multi_waits(nc)
    return nc


_NC = None


def _get_nc():
    global _NC
    if _NC is None:
        _NC = build_nc()
    return _NC


def _prep_inputs(x, padding_mask, wq_sw, wk_sw, wv_sw, wq_nw, wk_nw, wv_nw, out_w):
    """Host-side layout prep. Returns per-core input dicts."""
    xT = np.ascontiguousarray(x.transpose(0, 2, 1))          # [B, 512, L]
    xt = xT.reshape(B, NIC, 128, L).astype(NP_DT)
    # xns[p, 512c + 8n + b] = x[b, 2048+n, 128c+p]
    xns_f = x[:, L_S:, :].transpose(2, 1, 0)                 # [512, 64, 8]
    xns = np.ascontiguousarray(
        xns_f.reshape(NIC, 128, L_NS * 8)
    ).transpose(1, 0, 2).reshape(128, NIC * L_NS * 8)
    xns = np.ascontiguousarray(xns).astype(NP_DT)

    padf = np.concatenate(
        [padding_mask.astype(np.float32), np.ones((B, L_NS), np.float32)], axis=1
    )                                                        # [B, L]
    padp = np.zeros((B, NKC * 128), np.float32)
    padp[:, :L] = padf
    # padf_t[p, NKC*b + kc] = padp[b, 128*kc + p]
    padt = padp.reshape(B, NKC, 128).transpose(2, 0, 1).reshape(128, B * NKC)
    padt = np.ascontiguousarray(padt).astype(np.float32)

    in_maps = []
    for h in range(H):
        sl = slice(HD * h, HD * (h + 1))
        wkv = np.concatenate([wk_sw[:, sl], wv_sw[:, sl]], axis=1)   # [512, 128]
        wkv = wkv.reshape(NIC, 128, 128).transpose(1, 0, 2).reshape(128, NIC * 128)
        wqh = wq_sw[:, sl].reshape(NIC, 128, 64).transpose(1, 0, 2).reshape(
            128, NIC * 64
        )
        # nw[g, p, 768*nl + 192*c + j] = cat(wq_nw, wk_nw, wv_nw)[4g+nl, 128c+p, j]
        nwcat = np.concatenate(
            [wq_nw[:, :, sl], wk_nw[:, :, sl], wv_nw[:, :, sl]], axis=2
        )                                                            # [64, 512, 192]
        nwg = nwcat.reshape(16, 4, NIC, 128, 192).transpose(0, 3, 1, 2, 4)
        nwg = np.ascontiguousarray(nwg).reshape(16, 128, 3072)
        in_maps.append(
            dict(
                xt=xt,
                xns=xns,
                wkv=np.ascontiguousarray(wkv).astype(NP_DT),
                wq=np.ascontiguousarray(wqh).astype(NP_DT),
                nw=nwg.astype(NP_DT),
                ow=np.ascontiguousarray(out_w[sl, :]).astype(np.float32),
                padf=padt,
            )
        )
    return in_maps


def _reference_numpy(x, padding_mask, L_s, L_s_out, params):
    """Exact fallback (only used if inputs deviate from the expected pattern)."""
    def mix_linear(xx, Ls, sw, sb, nw, nb):
        out_s = xx[:, :Ls] @ sw + sb
        out_ns = np.einsum("bni,nio->bno", xx[:, Ls:], nw) + nb
        return np.concatenate([out_s, out_ns], axis=1)

    p = params
    Bb, Lx, d = x.shape
    hd = d // H
    K = mix_linear(x, L_s, p["wk_sw"], p["wk_sb"], p["wk_nw"], p["wk_nb"])
    V = mix_linear(x, L_s, p["wv_sw"], p["wv_sb"], p["wv_nw"], p["wv_nb"])
    K = K.reshape(Bb, Lx, H, hd).transpose(0, 2, 1, 3)
    V = V.reshape(Bb, Lx, H, hd).transpose(0, 2, 1, 3)
    if L_s_out < L_s:
        q_input = np.concatenate([x[:, L_s - L_s_out : L_s], x[:, L_s:]], axis=1)
    else:
        q_input = x
    L_q = L_s_out + (Lx - L_s)
    L_k = Lx
    Q = mix_linear(q_input, L_s_out, p["wq_sw"], p["wq_sb"], p["wq_nw"], p["wq_nb"])
    Q = Q.reshape(Bb, L_q, H, hd).transpose(0, 2, 1, 3)
    scores = np.einsum("bhqd,bhkd->bhqk", Q, K) / np.sqrt(hd).astype(np.float32)
    i = np.arange(L_q)[:, None]
    j = np.arange(L_k)[None, :]
    causal = j <= i + (L_k - L_q)
    pad = np.concatenate(
        [padding_mask.astype(bool), np.ones((Bb, Lx - L_s), bool)], axis=1
    )
    mask = causal[None, None] & pad[:, None, None, :]
    scores = np.where(mask, scores, -1e9)
    scores = scores - scores.max(axis=-1, keepdims=True)
    w = np.exp(scores)
    w = w / w.sum(axis=-1, keepdims=True)
    out = np.einsum("bhqk,bhkd->bhqd", w, V).transpose(0, 2, 1, 3).reshape(Bb, L_q, d)
    return (out @ p["out_w"] + p["out_b"]).astype(np.float32)


def kernel(
    x, padding_mask, L_s, L_s_out,
    wq_sw, wq_sb, wq_nw, wq_nb,
    wk_sw, wk_sb, wk_nw, wk_nb,
    wv_sw, wv_sb, wv_nw, wv_nb,
    out_w, out_b,
):
    x = np.asarray(x, np.float32)
    padding_mask = np.asarray(padding_mask)
    params = dict(
        wq_sw=np.asarray(wq_sw, np.float32), wq_sb=np.asarray(wq_sb, np.float32),
        wq_nw=np.asarray(wq_nw, np.float32), wq_nb=np.asarray(wq_nb, np.float32),
        wk_sw=np.asarray(wk_sw, np.float32), wk_sb=np.asarray(wk_sb, np.float32),
        wk_nw=np.asarray(wk_nw, np.float32), wk_nb=np.asarray(wk_nb, np.float32),
        wv_sw=np.asarray(wv_sw, np.float32), wv_sb=np.asarray(wv_sb, np.float32),
        wv_nw=np.asarray(wv_nw, np.float32), wv_nb=np.asarray(wv_nb, np.float32),
        out_w=np.asarray(out_w, np.float32), out_b=np.asarray(out_b, np.float32),
    )
    biases_zero = all(
        not np.any(params[k])
        for k in ["wq_sb", "wq_nb", "wk_sb", "wk_nb", "wv_sb", "wv_nb"]
    )
    if (
        int(L_s) != L_S or int(L_s_out) != L_SO or x.shape != (B, L, D)
        or not biases_zero
    ):
        return _reference_numpy(x, padding_mask, int(L_s), int(L_s_out), params)

    in_maps = _prep_inputs(
        x, padding_mask,
        params["wq_sw"], params["wk_sw"], params["wv_sw"],
        params["wq_nw"], params["wk_nw"], params["wv_nw"],
        params["out_w"],
    )
    nc = _get_nc()
    res = run_bass_kernel_spmd(
        nc, in_maps, core_ids=list(range(H)),
        trace=bool(os.environ.get("KERNEL_TRACE")),
    )
    global _LAST_RESULT
    _LAST_RESULT = res
    out = np.zeros((B, LQ, D), np.float32)
    for h in range(H):
        out += res.results[h]["out_p"]
    out += params["out_b"][None, None, :]
    return out
